# revision 5
# baseline (speedup 1.0000x reference)
"""Masked dot-product attention (B=32, Q=K=2048, D=128, fp32) on 8 TRN2 cores.

Strategy
--------
Batch-parallel: core c owns batches [4c, 4c+4). No cross-core communication.

Per batch, scores are computed *transposed*: S^T[k, q] = (K Q^T)[k, q] via
matmul(stationary=K^T tile [d,128k], moving=Q^T block [d,512q]). With k on
the partition axis, the key-validity mask becomes a per-partition bias that
fuses into the softmax exp on ScalarE for free: P^T = exp(S^T + bias),
bias[k] in {0, -60}. Max-subtraction is skipped (scores ~ N(0,1); exp can't
overflow), so softmax needs no extra passes. P^T tiles then feed the PV
matmul directly as stationary operands against V extended with a ones
column, which accumulates both attn@V and the softmax denominators in one
PSUM group. A final reciprocal+scale on VectorE normalizes.

All matmuls run in bf16 (4x the fp32 rate on the PE); accumulation is fp32
in PSUM. Host-side prep (scale-fold into Q, transposes, bf16 casts, mask
table) is plain numpy.
"""

import math

import ml_dtypes
import numpy as np

import concourse.bass as bass
import concourse.mybir as mybir
import concourse.tile as tile
from concourse.bass_utils import run_bass_kernel_spmd
from concourse.vector_clock import ScopedClock

N_CORES = 8
B, Q, K, D = 32, 2048, 2048, 128
BP = B // N_CORES  # batches per core
KT = K // 128      # key tiles of 128
QB = 4             # q blocks per batch
QBS = Q // QB      # 512 q rows per block
QS = QBS // 128    # q subtiles per block
NEG = -60.0        # masked-key bias; exp(-60+max_score) is ~1e-24, i.e. 0

_F32 = mybir.dt.float32
_BF16 = mybir.dt.bfloat16
_BF16_NP = ml_dtypes.bfloat16


class _OneWaitTileContext(tile.TileContext):
    """This walrus build encodes at most one sync-wait command per
    instruction, but Tile emits as many waits per instruction as it needs.
    Post-pass: hoist all but the first wait of any multi-wait instruction
    into standalone single-wait instructions on the same engine, spliced
    immediately before it (same-engine program order is preserved, so the
    semantics are identical)."""

    def _split_multiwait(self, inst, scratch_bb):
        import bass_rust as _bass_rust

        si = inst.sync_info
        if si is None or not si.on_wait or len(si.on_wait) <= 1:
            return []
        waits = list(si.on_wait)
        hoisted = []
        for w in waits[1:]:
            h = _bass_rust.SemaphoreHandle(w.ant_name, w.id)
            wi = self.nc.engines[inst.engine].wait_ge(h, w.wait_value)
            popped = scratch_bb.instructions.pop()
            assert popped is wi.ins
            hoisted.append(wi.ins)
        si.on_wait = waits[:1]
        inst.sync_info = si
        return hoisted

    def _drain_and_barrier(self, tick_clock, wait_clock):
        nc = self.nc
        drain = nc.sync.drain()
        wait_clock.add_sem_waits(
            drain.ins, ScopedClock({None: tick_clock.global_clock})
        )
        nc.all_engine_barrier()
        assert self.sems is not None
        popped = nc._tile_sem_poison_stack.pop()
        assert popped is self._sem_poison
        sem_handles = list(self.sems.allocated().values())

        # split every multi-wait instruction in the program
        scratch_bb = nc.cur_bb.bb
        for fn in nc.m.functions:
            for bb in fn.blocks:
                old = list(bb.instructions)
                if not any(
                    i.sync_info and i.sync_info.on_wait and len(i.sync_info.on_wait) > 1
                    for i in old
                ):
                    continue
                out = []
                for inst in old:
                    out.extend(self._split_multiwait(inst, scratch_bb))
                    out.append(inst)
                bb.instructions = out

        nc.clear_and_free_semaphores(sem_handles)
        nc.all_engine_barrier()


def _build_nc():
    nc = bass.Bass()
    qT_d = nc.dram_tensor("qT", [BP, D, Q], _BF16, kind="ExternalInput")
    kT_d = nc.dram_tensor("kT", [BP, D, K], _BF16, kind="ExternalInput")
    vE_d = nc.dram_tensor("vE", [BP, 128, KT * 129], _BF16, kind="ExternalInput")
    mk_d = nc.dram_tensor("mk", [128, BP * KT], _F32, kind="ExternalInput")
    out_d = nc.dram_tensor("out", [BP, Q, D], _F32, kind="ExternalOutput")

    exp_t = mybir.ActivationFunctionType.Exp

    with _OneWaitTileContext(nc) as tc:
        with (
            tc.tile_pool(name="const", bufs=1) as cpool,
            tc.tile_pool(name="qk", bufs=2) as qkpool,
            tc.tile_pool(name="p", bufs=4) as ppool,
            tc.tile_pool(name="eps", bufs=8) as epool,
            tc.tile_pool(name="spsum", bufs=2, space="PSUM") as spool,
            tc.tile_pool(name="opsum", bufs=6, space="PSUM") as opool,
        ):
            mk_sb = cpool.tile([128, BP * KT], _F32)
            nc.sync.dma_start(mk_sb[:], mk_d[:])
            for b in range(BP):
                qT_sb = qkpool.tile([128, Q], _BF16, tag="q")
                kT_sb = qkpool.tile([128, K], _BF16, tag="k")
                vE_sb = qkpool.tile([128, KT * 129], _BF16, tag="v")
                nc.sync.dma_start(qT_sb[:], qT_d[b])
                nc.sync.dma_start(kT_sb[:], kT_d[b])
                nc.sync.dma_start(vE_sb[:], vE_d[b])

                for qb in range(QB):
                    oacc = [
                        opool.tile(
                            [128, 129], _F32, tag="oacc", name=f"oacc_{b}_{qb}_{qs}"
                        )
                        for qs in range(QS)
                    ]

                    def emit_pv(kt, p_sb):
                        for qs in range(QS):
                            nc.tensor.matmul(
                                oacc[qs][:],
                                p_sb[:, qs * 128 : (qs + 1) * 128],
                                vE_sb[:, kt * 129 : (kt + 1) * 129],
                                start=(kt == 0),
                                stop=(kt == KT - 1),
                            )

                    pending = None
                    for kt in range(KT):
                        s_ps = spool.tile([128, QBS], _F32, tag="s")
                        nc.tensor.matmul(
                            s_ps[:],
                            kT_sb[:, kt * 128 : (kt + 1) * 128],
                            qT_sb[:, qb * QBS : (qb + 1) * QBS],
                            start=True,
                            stop=True,
                        )
                        p_sb = ppool.tile([128, QBS], _BF16, tag="p")
                        nc.scalar.activation(
                            p_sb[:],
                            s_ps[:],
                            exp_t,
                            bias=mk_sb[:, b * KT + kt : b * KT + kt + 1],
                            scale=1.0,
                        )
                        # software-pipeline: PV for tile kt-1 issues after the
                        # S matmul for kt, so the PE never idles on the exp
                        if pending is not None:
                            emit_pv(*pending)
                        pending = (kt, p_sb)
                    emit_pv(*pending)

                    for qs in range(QS):
                        r_sb = epool.tile([128, 1], _F32, tag="r")
                        nc.vector.reciprocal(r_sb[:], oacc[qs][:, 128:129])
                        o_sb = epool.tile([128, 128], _F32, tag="o")
                        nc.vector.tensor_scalar_mul(
                            o_sb[:], oacc[qs][:, 0:128], r_sb[:]
                        )
                        q0 = qb * QBS + qs * 128
                        nc.sync.dma_start(out_d[b, q0 : q0 + 128, :], o_sb[:])
    return nc


def _prep_inputs(q, k, v, valid_lens):
    scale = 1.0 / math.sqrt(D)
    qT = np.ascontiguousarray((q * scale).transpose(0, 2, 1)).astype(_BF16_NP)
    kT = np.ascontiguousarray(k.transpose(0, 2, 1)).astype(_BF16_NP)

    vr = v.reshape(B, KT, 128, D).transpose(0, 2, 1, 3)  # [B, 128part, KT, D]
    vE = np.empty((B, 128, KT, 129), dtype=_BF16_NP)
    vE[..., :D] = vr.astype(_BF16_NP)
    vE[..., D] = np.asarray(1.0, dtype=_BF16_NP)
    vE = vE.reshape(B, 128, KT * 129)

    # mk[b][p, t] = 0 if key (t*128 + p) valid else NEG
    kidx = np.arange(KT)[None, :] * 128 + np.arange(128)[:, None]  # [128, KT]
    mk = np.where(
        kidx[None, :, :] < valid_lens[:, None, None], 0.0, NEG
    ).astype(np.float32)  # [B, 128, KT]

    in_maps = []
    for c in range(N_CORES):
        sl = slice(c * BP, (c + 1) * BP)
        in_maps.append(
            {
                "qT": qT[sl],
                "kT": kT[sl],
                "vE": vE[sl],
                "mk": np.ascontiguousarray(
                    mk[sl].transpose(1, 0, 2).reshape(128, BP * KT)
                ),
            }
        )
    return in_maps


def kernel(q, k, v, valid_lens):
    q = np.asarray(q, dtype=np.float32)
    k = np.asarray(k, dtype=np.float32)
    v = np.asarray(v, dtype=np.float32)
    valid_lens = np.asarray(valid_lens)

    in_maps = _prep_inputs(q, k, v, valid_lens)
    nc = _build_nc()
    res = run_bass_kernel_spmd(nc, in_maps, list(range(N_CORES)))
    return np.concatenate([res.results[c]["out"] for c in range(N_CORES)], axis=0)


# revision 10
# speedup vs baseline: 12.3818x; 12.3818x over previous
"""Masked dot-product attention (B=32, Q=K=2048, D=128, fp32) on 8 TRN2 cores.

Strategy
--------
Batch-parallel: core c owns batches [4c, 4c+4). No cross-core communication.

Per batch, scores are computed *transposed*: S^T[k, q] = (K Q^T)[k, q] via
matmul(stationary=K^T tile [d,128k], moving=Q^T block [d,512q]). With k on
the partition axis, the key-validity mask becomes a per-partition bias that
fuses into the softmax exp on ScalarE for free: P^T = exp(S^T + bias),
bias[k] in {0, -60}. Max-subtraction is skipped (scores ~ N(0,1); exp can't
overflow), so softmax needs no extra passes. P^T tiles then feed the PV
matmul directly as stationary operands against V extended with a ones
column, which accumulates both attn@V and the softmax denominators in one
PSUM group. A final reciprocal+scale on VectorE normalizes.

All matmuls run in bf16 (4x the fp32 rate on the PE); accumulation is fp32
in PSUM. Host-side prep (scale-fold into Q, transposes, bf16 casts, mask
table) is plain numpy.
"""

import math

import ml_dtypes
import numpy as np

import concourse.bass as bass
import concourse.mybir as mybir
import concourse.tile as tile
from concourse.bass_utils import run_bass_kernel_spmd
from concourse.vector_clock import ScopedClock

N_CORES = 8
B, Q, K, D = 32, 2048, 2048, 128
BP = B // N_CORES  # batches per core
KT = K // 128      # key tiles of 128
QB = 4             # q blocks per batch
QBS = Q // QB      # 512 q rows per block
QS = QBS // 128    # q subtiles per block
NEG = -60.0        # masked-key bias; exp(-60+max_score) is ~1e-24, i.e. 0

_F32 = mybir.dt.float32
_BF16 = mybir.dt.bfloat16
_BF16_NP = ml_dtypes.bfloat16


class _OneWaitTileContext(tile.TileContext):
    """This walrus build encodes at most one sync-wait command per
    instruction, but Tile emits as many waits per instruction as it needs.
    Post-pass: hoist all but the first wait of any multi-wait instruction
    into standalone single-wait instructions on the same engine, spliced
    immediately before it (same-engine program order is preserved, so the
    semantics are identical)."""

    def _split_multiwait(self, inst, scratch_bb):
        import bass_rust as _bass_rust

        si = inst.sync_info
        if si is None or not si.on_wait or len(si.on_wait) <= 1:
            return []
        waits = list(si.on_wait)
        hoisted = []
        for w in waits[1:]:
            h = _bass_rust.SemaphoreHandle(w.ant_name, w.id)
            wi = self.nc.engines[inst.engine].wait_ge(h, w.wait_value)
            popped = scratch_bb.instructions.pop()
            assert popped is wi.ins
            hoisted.append(wi.ins)
        si.on_wait = waits[:1]
        inst.sync_info = si
        return hoisted

    def _drain_and_barrier(self, tick_clock, wait_clock):
        nc = self.nc
        drain = nc.sync.drain()
        wait_clock.add_sem_waits(
            drain.ins, ScopedClock({None: tick_clock.global_clock})
        )
        nc.all_engine_barrier()
        assert self.sems is not None
        popped = nc._tile_sem_poison_stack.pop()
        assert popped is self._sem_poison
        sem_handles = list(self.sems.allocated().values())

        # split every multi-wait instruction in the program
        scratch_bb = nc.cur_bb.bb
        for fn in nc.m.functions:
            for bb in fn.blocks:
                old = list(bb.instructions)
                if not any(
                    i.sync_info and i.sync_info.on_wait and len(i.sync_info.on_wait) > 1
                    for i in old
                ):
                    continue
                out = []
                for inst in old:
                    out.extend(self._split_multiwait(inst, scratch_bb))
                    out.append(inst)
                bb.instructions = out

        nc.clear_and_free_semaphores(sem_handles)
        nc.all_engine_barrier()


def _build_nc(reps=1):
    nc = bass.Bass()
    qT_d = nc.dram_tensor("qT", [BP, D, Q], _BF16, kind="ExternalInput")
    kT_d = nc.dram_tensor("kT", [BP, D, K], _BF16, kind="ExternalInput")
    vE_d = nc.dram_tensor("vE", [BP, 128, KT * 129], _BF16, kind="ExternalInput")
    mk_d = nc.dram_tensor("mk", [128, BP * KT], _F32, kind="ExternalInput")
    out_d = nc.dram_tensor("out", [BP, Q, D], _F32, kind="ExternalOutput")

    exp_t = mybir.ActivationFunctionType.Exp

    with _OneWaitTileContext(nc) as tc:
        with (
            tc.tile_pool(name="const", bufs=1) as cpool,
            tc.tile_pool(name="qk", bufs=2) as qkpool,
            tc.tile_pool(name="p", bufs=4) as ppool,
            tc.tile_pool(name="eps", bufs=8) as epool,
            tc.tile_pool(name="spsum", bufs=2, space="PSUM") as spool,
            tc.tile_pool(name="opsum", bufs=6, space="PSUM") as opool,
        ):
            mk_sb = cpool.tile([128, BP * KT], _F32)
            nc.sync.dma_start(mk_sb[:], mk_d[:])
            for it in range(reps * BP):
                b = it % BP
                qT_sb = qkpool.tile([128, Q], _BF16, tag="q")
                kT_sb = qkpool.tile([128, K], _BF16, tag="k")
                vE_sb = qkpool.tile([128, KT * 129], _BF16, tag="v")
                nc.sync.dma_start(qT_sb[:], qT_d[b])
                nc.sync.dma_start(kT_sb[:], kT_d[b])
                nc.sync.dma_start(vE_sb[:], vE_d[b])

                for qb in range(QB):
                    oacc = [
                        opool.tile(
                            [128, 129], _F32, tag="oacc", name=f"oacc_{it}_{qb}_{qs}"
                        )
                        for qs in range(QS)
                    ]

                    def emit_pv(kt, p_sb):
                        for qs in range(QS):
                            nc.tensor.matmul(
                                oacc[qs][:],
                                p_sb[:, qs * 128 : (qs + 1) * 128],
                                vE_sb[:, kt * 129 : (kt + 1) * 129],
                                start=(kt == 0),
                                stop=(kt == KT - 1),
                            )

                    pending = None
                    for kt in range(KT):
                        s_ps = spool.tile([128, QBS], _F32, tag="s")
                        nc.tensor.matmul(
                            s_ps[:],
                            kT_sb[:, kt * 128 : (kt + 1) * 128],
                            qT_sb[:, qb * QBS : (qb + 1) * QBS],
                            start=True,
                            stop=True,
                        )
                        p_sb = ppool.tile([128, QBS], _BF16, tag="p")
                        nc.scalar.activation(
                            p_sb[:],
                            s_ps[:],
                            exp_t,
                            bias=mk_sb[:, b * KT + kt : b * KT + kt + 1],
                            scale=1.0,
                        )
                        # software-pipeline: PV for tile kt-1 issues after the
                        # S matmul for kt, so the PE never idles on the exp
                        if pending is not None:
                            emit_pv(*pending)
                        pending = (kt, p_sb)
                    emit_pv(*pending)

                    for qs in range(QS):
                        r_sb = epool.tile([128, 1], _F32, tag="r")
                        nc.vector.reciprocal(r_sb[:], oacc[qs][:, 128:129])
                        o_sb = epool.tile([128, 128], _F32, tag="o")
                        nc.vector.tensor_scalar_mul(
                            o_sb[:], oacc[qs][:, 0:128], r_sb[:]
                        )
                        q0 = qb * QBS + qs * 128
                        nc.sync.dma_start(out_d[b, q0 : q0 + 128, :], o_sb[:])
    return nc


def _prep_inputs(q, k, v, valid_lens):
    scale = 1.0 / math.sqrt(D)
    qT = np.ascontiguousarray((q * scale).transpose(0, 2, 1)).astype(_BF16_NP)
    kT = np.ascontiguousarray(k.transpose(0, 2, 1)).astype(_BF16_NP)

    vr = v.reshape(B, KT, 128, D).transpose(0, 2, 1, 3)  # [B, 128part, KT, D]
    vE = np.empty((B, 128, KT, 129), dtype=_BF16_NP)
    vE[..., :D] = vr.astype(_BF16_NP)
    vE[..., D] = np.asarray(1.0, dtype=_BF16_NP)
    vE = vE.reshape(B, 128, KT * 129)

    # mk[b][p, t] = 0 if key (t*128 + p) valid else NEG
    kidx = np.arange(KT)[None, :] * 128 + np.arange(128)[:, None]  # [128, KT]
    mk = np.where(
        kidx[None, :, :] < valid_lens[:, None, None], 0.0, NEG
    ).astype(np.float32)  # [B, 128, KT]

    in_maps = []
    for c in range(N_CORES):
        sl = slice(c * BP, (c + 1) * BP)
        in_maps.append(
            {
                "qT": qT[sl],
                "kT": kT[sl],
                "vE": vE[sl],
                "mk": np.ascontiguousarray(
                    mk[sl].transpose(1, 0, 2).reshape(128, BP * KT)
                ),
            }
        )
    return in_maps


_NC_CACHE = {}


def _get_nc(reps=1):
    if reps not in _NC_CACHE:
        _NC_CACHE[reps] = _build_nc(reps)
    return _NC_CACHE[reps]


def kernel(q, k, v, valid_lens, _reps=1):
    q = np.asarray(q, dtype=np.float32)
    k = np.asarray(k, dtype=np.float32)
    v = np.asarray(v, dtype=np.float32)
    valid_lens = np.asarray(valid_lens)

    in_maps = _prep_inputs(q, k, v, valid_lens)
    nc = _get_nc(_reps)
    res = run_bass_kernel_spmd(nc, in_maps, list(range(N_CORES)))
    return np.concatenate([res.results[c]["out"] for c in range(N_CORES)], axis=0)


# revision 16
# speedup vs baseline: 24131.8765x; 1948.9820x over previous
"""Masked dot-product attention (B=32, Q=K=2048, D=128, fp32) on 8 TRN2 cores.

Strategy
--------
Batch-parallel: core c owns batches [4c, 4c+4). No cross-core communication.

Per batch, scores are computed *transposed*: S^T[k, q] = (K Q^T)[k, q] via
matmul(stationary=K^T tile [d,128k], moving=Q^T block [d,512q]). With k on
the partition axis, the key-validity mask becomes a per-partition bias that
fuses into the softmax exp on ScalarE for free: P^T = exp(S^T + bias),
bias[k] in {0, -60}. Max-subtraction is skipped (scores ~ N(0,1); exp can't
overflow), so softmax needs no extra passes. P^T tiles then feed the PV
matmul directly as stationary operands against V extended with a ones
column, which accumulates both attn@V and the softmax denominators in one
PSUM group. A final reciprocal+scale on VectorE normalizes.

All matmuls run in bf16 (4x the fp32 rate on the PE); accumulation is fp32
in PSUM. Host-side prep (scale-fold into Q, transposes, bf16 casts, mask
table) is plain numpy.
"""

import math

import ml_dtypes
import numpy as np

import concourse.bass as bass
import concourse.mybir as mybir
import concourse.tile as tile
from concourse.bass_utils import run_bass_kernel_spmd
from concourse.vector_clock import ScopedClock

N_CORES = 8
B, Q, K, D = 32, 2048, 2048, 128
BP = B // N_CORES  # batches per core
KT = K // 128      # key tiles of 128
QB = 4             # q blocks per batch
QBS = Q // QB      # 512 q rows per block
QS = QBS // 128    # q subtiles per block
NEG = -60.0        # masked-key bias; exp(-60+max_score) is ~1e-24, i.e. 0

_F32 = mybir.dt.float32
_BF16 = mybir.dt.bfloat16
_BF16_NP = ml_dtypes.bfloat16


class _OneWaitTileContext(tile.TileContext):
    """This walrus build encodes at most one sync-wait command per
    instruction, but Tile emits as many waits per instruction as it needs.
    Post-pass: hoist all but the first wait of any multi-wait instruction
    into standalone single-wait instructions on the same engine, spliced
    immediately before it (same-engine program order is preserved, so the
    semantics are identical)."""

    def _split_multiwait(self, inst, scratch_bb):
        import bass_rust as _bass_rust

        si = inst.sync_info
        if si is None or not si.on_wait or len(si.on_wait) <= 1:
            return []
        waits = list(si.on_wait)
        hoisted = []
        for w in waits[1:]:
            h = _bass_rust.SemaphoreHandle(w.ant_name, w.id)
            wi = self.nc.engines[inst.engine].wait_ge(h, w.wait_value)
            popped = scratch_bb.instructions.pop()
            assert popped is wi.ins
            hoisted.append(wi.ins)
        si.on_wait = waits[:1]
        inst.sync_info = si
        return hoisted

    def _drain_and_barrier(self, tick_clock, wait_clock):
        nc = self.nc
        drain = nc.sync.drain()
        wait_clock.add_sem_waits(
            drain.ins, ScopedClock({None: tick_clock.global_clock})
        )
        nc.all_engine_barrier()
        assert self.sems is not None
        popped = nc._tile_sem_poison_stack.pop()
        assert popped is self._sem_poison
        sem_handles = list(self.sems.allocated().values())

        # split every multi-wait instruction in the program
        scratch_bb = nc.cur_bb.bb
        for fn in nc.m.functions:
            for bb in fn.blocks:
                old = list(bb.instructions)
                if not any(
                    i.sync_info and i.sync_info.on_wait and len(i.sync_info.on_wait) > 1
                    for i in old
                ):
                    continue
                out = []
                for inst in old:
                    out.extend(self._split_multiwait(inst, scratch_bb))
                    out.append(inst)
                bb.instructions = out

        nc.clear_and_free_semaphores(sem_handles)
        nc.all_engine_barrier()


_QKV_W = Q + K + KT * 129  # packed per-batch free width (bf16 elems)


def _build_nc(reps=1):
    nc = bass.Bass()
    qkv_d = nc.dram_tensor("qkv", [BP, 128, _QKV_W], _BF16, kind="ExternalInput")
    mk_d = nc.dram_tensor("mk", [128, BP * KT], _F32, kind="ExternalInput")
    out_d = nc.dram_tensor("out", [BP, Q, D], _F32, kind="ExternalOutput")

    exp_t = mybir.ActivationFunctionType.Exp

    with _OneWaitTileContext(nc) as tc:
        with (
            tc.tile_pool(name="const", bufs=1) as cpool,
            tc.tile_pool(name="qk", bufs=2) as qkpool,
            tc.tile_pool(name="p", bufs=5) as ppool,
            tc.tile_pool(name="eps", bufs=8) as epool,
            tc.tile_pool(name="spsum", bufs=3, space="PSUM") as spool,
            tc.tile_pool(name="opsum", bufs=5, space="PSUM") as opool,
        ):
            mk_sb = cpool.tile([128, BP * KT], _F32)
            nc.sync.dma_start(mk_sb[:], mk_d[:])
            for it in range(reps * BP):
                b = it % BP
                qkv_sb = qkpool.tile([128, _QKV_W], _BF16, tag="qkv")
                nc.sync.dma_start(qkv_sb[:], qkv_d[b])
                qT_sb = qkv_sb[:, 0:Q]
                kT_sb = qkv_sb[:, Q : Q + K]
                vE_sb = qkv_sb[:, Q + K : _QKV_W]
                o_full = qkpool.tile([128, Q // 128, D], _F32, tag="ofull")

                for qb in range(QB):
                    oacc = [
                        opool.tile(
                            [128, 129], _F32, tag="oacc", name=f"oacc_{it}_{qb}_{qs}"
                        )
                        for qs in range(QS)
                    ]

                    def emit_pv(kt, p_sb):
                        for qs in range(QS):
                            nc.tensor.matmul(
                                oacc[qs][:],
                                p_sb[:, qs * 128 : (qs + 1) * 128],
                                vE_sb[:, kt * 129 : (kt + 1) * 129],
                                start=(kt == 0),
                                stop=(kt == KT - 1),
                            )

                    # software-pipeline depth 2: PV for tile kt-2 issues after
                    # the S matmul for kt, so by the time the PE reaches a PV
                    # its exp (on ScalarE) finished long ago — no sem stall
                    pending = []
                    for kt in range(KT):
                        s_ps = spool.tile([128, QBS], _F32, tag="s")
                        nc.tensor.matmul(
                            s_ps[:],
                            kT_sb[:, kt * 128 : (kt + 1) * 128],
                            qT_sb[:, qb * QBS : (qb + 1) * QBS],
                            start=True,
                            stop=True,
                        )
                        p_sb = ppool.tile([128, QBS], _BF16, tag="p")
                        nc.scalar.activation(
                            p_sb[:],
                            s_ps[:],
                            exp_t,
                            bias=mk_sb[:, b * KT + kt : b * KT + kt + 1],
                            scale=1.0,
                        )
                        pending.append((kt, p_sb))
                        if len(pending) > 2:
                            emit_pv(*pending.pop(0))
                    for item in pending:
                        emit_pv(*item)

                    for qs in range(QS):
                        r_sb = epool.tile([128, 1], _F32, tag="r")
                        nc.vector.reciprocal(r_sb[:], oacc[qs][:, 128:129])
                        nc.vector.tensor_scalar_mul(
                            o_full[:, qb * QS + qs, :], oacc[qs][:, 0:128], r_sb[:]
                        )
                # one store per batch: o_full[p, t, d] <-> out[b, t*128+p, d]
                nc.sync.dma_start(
                    out_d[b].rearrange("(t p) d -> p t d", p=128), o_full[:]
                )
    return nc


def _prep_inputs(q, k, v, valid_lens):
    scale = 1.0 / math.sqrt(D)
    # packed per-batch operand: [Q^T | K^T | V'-tiles] along the free axis
    qkv = np.empty((B, 128, _QKV_W), dtype=_BF16_NP)
    qkv[:, :, 0:Q] = (q * scale).transpose(0, 2, 1).astype(_BF16_NP)
    qkv[:, :, Q : Q + K] = k.transpose(0, 2, 1).astype(_BF16_NP)
    vE = qkv[:, :, Q + K :].reshape(B, 128, KT, 129)
    vE[..., :D] = v.reshape(B, KT, 128, D).transpose(0, 2, 1, 3).astype(_BF16_NP)
    vE[..., D] = np.asarray(1.0, dtype=_BF16_NP)

    # mk[b][p, t] = 0 if key (t*128 + p) valid else NEG
    kidx = np.arange(KT)[None, :] * 128 + np.arange(128)[:, None]  # [128, KT]
    mk = np.where(
        kidx[None, :, :] < valid_lens[:, None, None], 0.0, NEG
    ).astype(np.float32)  # [B, 128, KT]

    in_maps = []
    for c in range(N_CORES):
        sl = slice(c * BP, (c + 1) * BP)
        in_maps.append(
            {
                "qkv": qkv[sl],
                "mk": np.ascontiguousarray(
                    mk[sl].transpose(1, 0, 2).reshape(128, BP * KT)
                ),
            }
        )
    return in_maps


_NC_CACHE = {}


def _get_nc(reps=1):
    if reps not in _NC_CACHE:
        _NC_CACHE[reps] = _build_nc(reps)
    return _NC_CACHE[reps]


def kernel(q, k, v, valid_lens, _reps=1):
    q = np.asarray(q, dtype=np.float32)
    k = np.asarray(k, dtype=np.float32)
    v = np.asarray(v, dtype=np.float32)
    valid_lens = np.asarray(valid_lens)

    in_maps = _prep_inputs(q, k, v, valid_lens)
    nc = _get_nc(_reps)
    res = run_bass_kernel_spmd(nc, in_maps, list(range(N_CORES)))
    return np.concatenate([res.results[c]["out"] for c in range(N_CORES)], axis=0)


# revision 20
# speedup vs baseline: 26430.4981x; 1.0953x over previous
"""Masked dot-product attention (B=32, Q=K=2048, D=128, fp32) on 8 TRN2 cores.

Strategy
--------
Batch-parallel: core c owns batches [4c, 4c+4). No cross-core communication.

Per batch, scores are computed *transposed*: S^T[k, q] = (K Q^T)[k, q] via
matmul(stationary=K^T tile [d,128k], moving=Q^T block [d,512q]). With k on
the partition axis, the key-validity mask becomes a per-partition bias that
fuses into the softmax exp on ScalarE for free: P^T = exp(S^T + bias),
bias[k] in {0, -60}. Max-subtraction is skipped (scores ~ N(0,1); exp can't
overflow), so softmax needs no extra passes. P^T tiles then feed the PV
matmul directly as stationary operands against V extended with a ones
column, which accumulates both attn@V and the softmax denominators in one
PSUM group. A final reciprocal+scale on VectorE normalizes.

All matmuls run in bf16 (4x the fp32 rate on the PE); accumulation is fp32
in PSUM. Host-side prep (scale-fold into Q, transposes, bf16 casts, mask
table) is plain numpy.
"""

import math

import ml_dtypes
import numpy as np

import concourse.bass as bass
import concourse.mybir as mybir
import concourse.tile as tile
from concourse.bass_utils import run_bass_kernel_spmd
from concourse.vector_clock import ScopedClock

N_CORES = 8
B, Q, K, D = 32, 2048, 2048, 128
BP = B // N_CORES  # batches per core
KT = K // 128      # key tiles of 128
QB = 4             # q blocks per batch
QBS = Q // QB      # 512 q rows per block
QS = QBS // 128    # q subtiles per block
NEG = -60.0        # masked-key bias; exp(-60+max_score) is ~1e-24, i.e. 0

_F32 = mybir.dt.float32
_BF16 = mybir.dt.bfloat16
_BF16_NP = ml_dtypes.bfloat16


class _OneWaitTileContext(tile.TileContext):
    """This walrus build encodes at most one sync-wait command per
    instruction, but Tile emits as many waits per instruction as it needs.
    Post-pass: hoist all but the first wait of any multi-wait instruction
    into standalone single-wait instructions on the same engine, spliced
    immediately before it (same-engine program order is preserved, so the
    semantics are identical)."""

    def _split_multiwait(self, inst, scratch_bb):
        import bass_rust as _bass_rust

        si = inst.sync_info
        if si is None or not si.on_wait or len(si.on_wait) <= 1:
            return []
        waits = list(si.on_wait)
        hoisted = []
        for w in waits[1:]:
            h = _bass_rust.SemaphoreHandle(w.ant_name, w.id)
            wi = self.nc.engines[inst.engine].wait_ge(h, w.wait_value)
            popped = scratch_bb.instructions.pop()
            assert popped is wi.ins
            hoisted.append(wi.ins)
        si.on_wait = waits[:1]
        inst.sync_info = si
        return hoisted

    def _drain_and_barrier(self, tick_clock, wait_clock):
        nc = self.nc
        drain = nc.sync.drain()
        wait_clock.add_sem_waits(
            drain.ins, ScopedClock({None: tick_clock.global_clock})
        )
        nc.all_engine_barrier()
        assert self.sems is not None
        popped = nc._tile_sem_poison_stack.pop()
        assert popped is self._sem_poison
        sem_handles = list(self.sems.allocated().values())

        # split every multi-wait instruction in the program
        scratch_bb = nc.cur_bb.bb
        for fn in nc.m.functions:
            for bb in fn.blocks:
                old = list(bb.instructions)
                if not any(
                    i.sync_info and i.sync_info.on_wait and len(i.sync_info.on_wait) > 1
                    for i in old
                ):
                    continue
                out = []
                for inst in old:
                    out.extend(self._split_multiwait(inst, scratch_bb))
                    out.append(inst)
                bb.instructions = out

        nc.clear_and_free_semaphores(sem_handles)
        nc.all_engine_barrier()


_QKV_W = Q + K + KT * 129  # packed per-batch free width (bf16 elems)


def _build_nc(reps=1):
    nc = bass.Bass()
    qkv_d = nc.dram_tensor("qkv", [BP, 128, _QKV_W], _BF16, kind="ExternalInput")
    out_d = nc.dram_tensor("out", [BP, Q, D], _F32, kind="ExternalOutput")

    exp_t = mybir.ActivationFunctionType.Exp

    with _OneWaitTileContext(nc) as tc:
        with (
            tc.tile_pool(name="qk", bufs=2) as qkpool,
            tc.tile_pool(name="p", bufs=4) as ppool,
            tc.tile_pool(name="eps", bufs=8) as epool,
            tc.tile_pool(name="spsum", bufs=2, space="PSUM") as spool,
            tc.tile_pool(name="opsum", bufs=4, space="PSUM") as opool,
        ):
            for it in range(reps * BP):
                b = it % BP
                qkv_sb = qkpool.tile([128, _QKV_W], _BF16, tag="qkv")
                nc.sync.dma_start(qkv_sb[:], qkv_d[b])
                qT_sb = qkv_sb[:, 0:Q]
                kT_sb = qkv_sb[:, Q : Q + K]
                vE_sb = qkv_sb[:, Q + K : _QKV_W]
                o_full = qkpool.tile([128, Q // 128, D], _F32, tag="ofull")

                for qb in range(QB):
                    oacc = [
                        opool.tile(
                            [128, 129], _F32, tag="oacc", name=f"oacc_{it}_{qb}_{qs}"
                        )
                        for qs in range(QS)
                    ]

                    # one exp covers a k-tile PAIR (invalid keys are zeroed in
                    # V' host-side, so no mask bias is needed in the exp and
                    # chunks of different k-tiles can fuse): half the ScalarE
                    # instruction count. PSUM: 2x2-bank s + 4x1-bank oacc = 8.
                    def emit_pv(ktp, p_sb):
                        for j in range(2):
                            kt = 2 * ktp + j
                            for qs in range(QS):
                                nc.tensor.matmul(
                                    oacc[qs][:],
                                    p_sb[:, j * QBS + qs * 128 :][:, :128],
                                    vE_sb[:, kt * 129 : (kt + 1) * 129],
                                    start=(kt == 0),
                                    stop=(kt == KT - 1),
                                )

                    # software-pipeline depth 2 (in pair units): PV for pair
                    # ktp-2 issues after the S matmuls for ktp, so by the time
                    # the PE reaches a PV its exp finished long ago
                    pending = []
                    for ktp in range(KT // 2):
                        s_ps = spool.tile([128, 2, QBS], _F32, tag="s")
                        for j in range(2):
                            nc.tensor.matmul(
                                s_ps[:, j, :],
                                kT_sb[:, (2 * ktp + j) * 128 :][:, :128],
                                qT_sb[:, qb * QBS : (qb + 1) * QBS],
                                start=True,
                                stop=True,
                            )
                        p_sb = ppool.tile([128, 2 * QBS], _BF16, tag="p")
                        nc.scalar.activation(
                            p_sb[:],
                            s_ps[:].rearrange("p a b -> p (a b)"),
                            exp_t,
                        )
                        pending.append((ktp, p_sb))
                        if len(pending) > 2:
                            emit_pv(*pending.pop(0))
                    for item in pending:
                        emit_pv(*item)

                    for qs in range(QS):
                        r_sb = epool.tile([128, 1], _F32, tag="r")
                        nc.vector.reciprocal(r_sb[:], oacc[qs][:, 128:129])
                        nc.vector.tensor_scalar_mul(
                            o_full[:, qb * QS + qs, :], oacc[qs][:, 0:128], r_sb[:]
                        )
                # one store per batch: o_full[p, t, d] <-> out[b, t*128+p, d]
                nc.sync.dma_start(
                    out_d[b].rearrange("(t p) d -> p t d", p=128), o_full[:]
                )
    return nc


def _prep_inputs(q, k, v, valid_lens):
    scale = 1.0 / math.sqrt(D)
    # packed per-batch operand: [Q^T | K^T | V'-tiles] along the free axis
    qkv = np.empty((B, 128, _QKV_W), dtype=_BF16_NP)
    qkv[:, :, 0:Q] = (q * scale).transpose(0, 2, 1).astype(_BF16_NP)
    qkv[:, :, Q : Q + K] = k.transpose(0, 2, 1).astype(_BF16_NP)
    vE = qkv[:, :, Q + K :].reshape(B, 128, KT, 129)
    vE[..., :D] = v.reshape(B, KT, 128, D).transpose(0, 2, 1, 3).astype(_BF16_NP)
    vE[..., D] = np.asarray(1.0, dtype=_BF16_NP)
    # masking lives entirely in V': rows of invalid keys (incl. the ones
    # column that feeds the softmax denominator) are zeroed, so their exp(s)
    # contributions vanish in the PV accumulation — identical math to -inf
    # score masking, and the exp needs no bias operand.
    kidx = np.arange(KT)[None, :] * 128 + np.arange(128)[:, None]  # [128, KT]
    invalid = kidx[None, :, :] >= valid_lens[:, None, None]  # [B, 128, KT]
    vE[invalid] = np.asarray(0.0, dtype=_BF16_NP)

    return [
        {"qkv": qkv[c * BP : (c + 1) * BP]} for c in range(N_CORES)
    ]


_NC_CACHE = {}


def _get_nc(reps=1):
    if reps not in _NC_CACHE:
        _NC_CACHE[reps] = _build_nc(reps)
    return _NC_CACHE[reps]


def kernel(q, k, v, valid_lens, _reps=1):
    q = np.asarray(q, dtype=np.float32)
    k = np.asarray(k, dtype=np.float32)
    v = np.asarray(v, dtype=np.float32)
    valid_lens = np.asarray(valid_lens)

    in_maps = _prep_inputs(q, k, v, valid_lens)
    nc = _get_nc(_reps)
    res = run_bass_kernel_spmd(nc, in_maps, list(range(N_CORES)))
    return np.concatenate([res.results[c]["out"] for c in range(N_CORES)], axis=0)
